# revision 6
# baseline (speedup 1.0000x reference)
"""Trainium2 Bass kernel for CrossAttentionConditionInjection.

Math: the attention keys/values come from a single condition token broadcast
across the sequence, so the scores are constant along the key axis; softmax is
exactly uniform and the attention output collapses to

    out[b, s, :] = (condition[b] @ Wv.T + bv) @ Wo.T + bo      (for every s)

independent of hidden_states / Wq / Wk / q entirely.  The kernel computes the
two 1024x1024 mat-vecs in fp32 on the PE and broadcast-writes the resulting
row across its sequence shard.

Sharding: flat (b*s) rows [4096] split into 8 shards of 512 rows; cores 0-3
serve batch 0, cores 4-7 batch 1.  Each core receives its batch's condition
row (transposed into [128, 8] so the contraction dim sits on partitions), the
full Wv.T / Wo.T in a [128, 8*1024] chunk-major layout, and the biases.

TRN2 constraint baked into the structure: a Matmult can carry at most ONE
sync wait (it lowers into the single-wait LDWEIGHTS struct), so condT/bvT are
packed into the tail of the wv DMA (one semaphore lane covers everything
stage 1 touches) and a throwaway 1x1 matmul absorbs the wo DMA lane before
stage 2's first real matmul.
"""

import numpy as np
from contextlib import ExitStack

import concourse.bass as bass
import concourse.bacc as bacc
import concourse.mybir as mybir
import concourse.tile as tile
from concourse.bass_utils import run_bass_kernel_spmd

B, S, D = 2, 2048, 1024
NCORES = 8
ROWS_PER_CORE = (B * S) // NCORES  # 512
KC = D // 128  # 8 contraction chunks
WV_W = KC * D + 2 * KC  # wv chunks + condT [128,8] + bvT [128,8]

_cache = {}


def _build():
    f32 = mybir.dt.float32
    nc = bacc.Bacc()

    wvp = nc.dram_tensor("wvp", [128, WV_W], f32, kind="ExternalInput")
    wot = nc.dram_tensor("wot", [128, KC * D], f32, kind="ExternalInput")
    bo = nc.dram_tensor("bo", [1, D], f32, kind="ExternalInput")
    y = nc.dram_tensor("y", [ROWS_PER_CORE, D], f32, kind="ExternalOutput")

    with tile.TileContext(nc) as tc, ExitStack() as ctx:
        wv_pool = ctx.enter_context(tc.tile_pool(name="wv", bufs=1))
        wo_pool = ctx.enter_context(tc.tile_pool(name="wo", bufs=1))
        small = ctx.enter_context(tc.tile_pool(name="small", bufs=1))
        outp = ctx.enter_context(tc.tile_pool(name="outp", bufs=1))
        psum1 = ctx.enter_context(
            tc.tile_pool(name="psum1", bufs=2, space=bass.MemorySpace.PSUM)
        )
        psum2 = ctx.enter_context(
            tc.tile_pool(name="psum2", bufs=2, space=bass.MemorySpace.PSUM)
        )
        psumd = ctx.enter_context(
            tc.tile_pool(name="psumd", bufs=1, space=bass.MemorySpace.PSUM)
        )

        ones1x128 = small.tile([1, 128], f32)
        nc.vector.memset(ones1x128[:], 1.0)

        # ---- bulk loads: wv (+condT +bvT packed at the tail), wo, bo ----
        wv_sb = wv_pool.tile([128, WV_W], f32)
        nc.sync.dma_start(wv_sb[:], wvp[:])
        wo_sb = wo_pool.tile([128, KC * D], f32)
        nc.sync.dma_start(wo_sb[:], wot[:])
        bo_sb = small.tile([1, D], f32)
        nc.sync.dma_start(bo_sb[:], bo[:])

        condT = wv_sb[:, KC * D : KC * D + KC]
        # stage bvT onto DVE once so later adds carry no DMA-lane wait
        bvT_sb = small.tile([128, KC], f32)
        nc.vector.tensor_copy(bvT_sb[:], wv_sb[:, KC * D + KC : KC * D + 2 * KC])

        # ---- stage 1: v1T[p, j] = v1[j*128 + p],  v1 = cond @ Wv.T + bv ----
        v1T_sb = small.tile([128, KC], f32)
        for j in range(KC):
            acc = psum1.tile([128, 1], f32, tag="acc1")
            for k in range(KC):
                nc.tensor.matmul(
                    acc[:],
                    wv_sb[:, k * D + j * 128 : k * D + (j + 1) * 128],
                    condT[:, k : k + 1],
                    start=(k == 0),
                    stop=(k == KC - 1),
                )
            nc.vector.tensor_add(v1T_sb[:, j : j + 1], acc[:], bvT_sb[:, j : j + 1])

        # ---- broadcast v1T columns across the free dim for stage-2 lhsT ----
        v1bc_sb = small.tile([128, D], f32)
        for k in range(KC):
            nc.vector.tensor_copy(
                v1bc_sb[:, k * 128 : (k + 1) * 128],
                v1T_sb[:, k : k + 1].broadcast_to([128, 128]),
            )

        # throwaway matmul so PE observes wo's DMA lane before the real
        # stage-2 matmul (which already carries its one allowed wait on DVE)
        scr = psumd.tile([1, 1], f32)
        nc.tensor.matmul(scr[:], wo_sb[:, 0:1], wo_sb[:, 0:1], start=True, stop=True)

        # ---- stage 2: out row (broadcast across partitions) = v1 @ Wo.T + bo ----
        out_sb = outp.tile([128, D], f32)
        for n in range(2):
            ns = slice(n * 512, (n + 1) * 512)
            acc = psum2.tile([128, 512], f32, tag="acc2")
            for k in range(KC):
                nc.tensor.matmul(
                    acc[:],
                    v1bc_sb[:, k * 128 : (k + 1) * 128],
                    wo_sb[:, k * D + n * 512 : k * D + (n + 1) * 512],
                    start=(k == 0),
                    stop=False,
                )
            nc.tensor.matmul(
                acc[:], ones1x128[:], bo_sb[:, ns], start=False, stop=True
            )
            nc.vector.tensor_copy(out_sb[:, ns], acc[:])

        # ---- broadcast-write the row across this core's 512 output rows ----
        for t in range(ROWS_PER_CORE // 128):
            nc.sync.dma_start(y[t * 128 : (t + 1) * 128, :], out_sb[:])

    nc.compile()
    return nc


def _prep_inputs(condition, Wv, bv, Wo, bo):
    cond = np.asarray(condition, np.float32)
    wvt = np.asarray(Wv, np.float32).T.reshape(KC, 128, D).transpose(1, 0, 2)
    wvt = wvt.reshape(128, KC * D)
    wot = np.ascontiguousarray(
        np.asarray(Wo, np.float32).T.reshape(KC, 128, D).transpose(1, 0, 2).reshape(128, KC * D)
    )
    bvT = np.asarray(bv, np.float32).reshape(KC, 128).T  # [128, KC]
    bo_ = np.ascontiguousarray(np.asarray(bo, np.float32).reshape(1, D))
    wvp = []
    for b in range(B):
        condT = cond[b].reshape(KC, 128).T  # [128, KC]
        wvp.append(
            np.ascontiguousarray(np.concatenate([wvt, condT, bvT], axis=1))
        )
    in_maps = []
    for i in range(NCORES):
        b = i // (NCORES // B)
        in_maps.append({"wvp": wvp[b], "wot": wot, "bo": bo_})
    return in_maps


def _run(in_maps, **kwargs):
    if "nc" not in _cache:
        _cache["nc"] = _build()
    return run_bass_kernel_spmd(
        _cache["nc"], in_maps, core_ids=list(range(NCORES)), **kwargs
    )


def kernel(hidden_states, condition, Wq, bq, Wk, bk, Wv, bv, Wo, bo):
    in_maps = _prep_inputs(condition, Wv, bv, Wo, bo)
    res = _run(in_maps)
    out = np.concatenate(
        [np.asarray(res.results[i]["y"]) for i in range(NCORES)], axis=0
    )
    return np.ascontiguousarray(out.reshape(B, S, D))


# revision 8
# speedup vs baseline: 1.1478x; 1.1478x over previous
"""Trainium2 Bass kernel for CrossAttentionConditionInjection.

Math: the attention keys/values come from a single condition token broadcast
across the sequence, so the scores are constant along the key axis; softmax is
exactly uniform and the attention output collapses to

    out[b, s, :] = (condition[b] @ Wv.T + bv) @ Wo.T + bo      (for every s)

independent of hidden_states / Wq / Wk / q entirely.  The kernel computes the
two 1024x1024 mat-vecs in fp32 and broadcast-writes the resulting row across
its sequence shard.

Sharding: flat (b*s) rows [4096] split into 8 shards of 512 rows; cores 0-3
serve batch 0, cores 4-7 batch 1.

Engine split (fp32 streams through the PE at ~4 cycles/column, so mat-vec
work is kept off the PE where possible):
  stage 1 (v1 = cond @ Wv.T):  DVE per-partition tensor_scalar multiplies
      (contraction dim on partitions) + tree add -> partial[128, 1024]
  partition-reduce + broadcast: PE ones-matmul  out[m,n] = sum_p partial[p,n]
      -> v1 replicated across all 128 partitions; bv added via a K=1
      rank-1 matmul with a ones row.
  stage 2 (row = v1 @ Wo.T + bo): PE broadcast-matmul over Wo.T chunks
      (accumulating over k), bias via ones-row matmul, all 128 output
      partitions = the row.
  output: 4x [128, 1024] DMA broadcast-writes.
"""

import numpy as np
from contextlib import ExitStack

import concourse.bass as bass
import concourse.bacc as bacc
import concourse.mybir as mybir
import concourse.tile as tile
from concourse.bass_utils import run_bass_kernel_spmd

B, S, D = 2, 2048, 1024
NCORES = 8
ROWS_PER_CORE = (B * S) // NCORES  # 512
KC = D // 128  # 8 contraction chunks
WV_W = KC * D + 2 * KC  # wv chunks + condT [128,8] + bvT [128,8] packed at the tail
WO_CHUNKS = 4

_cache = {}


def _build():
    f32 = mybir.dt.float32
    nc = bacc.Bacc()

    wvp = nc.dram_tensor("wvp", [128, WV_W], f32, kind="ExternalInput")
    wot = nc.dram_tensor("wot", [128, KC * D], f32, kind="ExternalInput")
    bo = nc.dram_tensor("bo", [1, D], f32, kind="ExternalInput")
    y = nc.dram_tensor("y", [ROWS_PER_CORE, D], f32, kind="ExternalOutput")

    with tile.TileContext(nc) as tc, ExitStack() as ctx:
        wv_pool = ctx.enter_context(tc.tile_pool(name="wv", bufs=1))
        wo_pool = ctx.enter_context(tc.tile_pool(name="wo", bufs=1))
        small = ctx.enter_context(tc.tile_pool(name="small", bufs=1))
        tmpp = ctx.enter_context(tc.tile_pool(name="tmpp", bufs=1))
        outp = ctx.enter_context(tc.tile_pool(name="outp", bufs=1))
        psumv = ctx.enter_context(
            tc.tile_pool(name="psumv", bufs=2, space=bass.MemorySpace.PSUM)
        )
        psum2 = ctx.enter_context(
            tc.tile_pool(name="psum2", bufs=2, space=bass.MemorySpace.PSUM)
        )

        ones1x128 = small.tile([1, 128], f32)
        nc.vector.memset(ones1x128[:], 1.0)
        ones_col = small.tile([128, 1], f32)
        nc.vector.memset(ones_col[:], 1.0)

        # ---- bulk loads ----
        wv_sb = wv_pool.tile([128, WV_W], f32)
        nc.sync.dma_start(wv_sb[:], wvp[:])
        wo_sb = wo_pool.tile([128, KC * D], f32)
        wo_c = (KC * D) // WO_CHUNKS
        for c in range(WO_CHUNKS):
            nc.sync.dma_start(
                wo_sb[:, c * wo_c : (c + 1) * wo_c], wot[:, c * wo_c : (c + 1) * wo_c]
            )
        bo_sb = small.tile([1, D], f32)
        nc.sync.dma_start(bo_sb[:], bo[:])

        condT = wv_sb[:, KC * D : KC * D + KC]
        bvT = wv_sb[:, KC * D + KC : KC * D + 2 * KC]

        # ---- stage 1 on DVE: partial[p, c] = sum_k WvT[k*128+p, c] * cond[k*128+p]
        tmp = tmpp.tile([128, KC * D], f32)
        for k in range(KC):
            nc.vector.tensor_scalar_mul(
                tmp[:, k * D : (k + 1) * D],
                wv_sb[:, k * D : (k + 1) * D],
                condT[:, k : k + 1],
            )
        h = (KC * D) // 2
        nc.vector.tensor_add(tmp[:, :h], tmp[:, :h], tmp[:, h:])
        nc.vector.tensor_add(tmp[:, : h // 2], tmp[:, : h // 2], tmp[:, h // 2 : h])
        partial = tmp[:, :D]
        nc.vector.tensor_add(partial, partial, tmp[:, D : 2 * D])

        # ---- partition-reduce on PE, landing v1T on partitions:
        #      out[m, 0] = sum_p partial[p, j*128+m] = v1[j*128+m]
        v1T_sb = small.tile([128, KC], f32)
        for j in range(KC):
            acc = psumv.tile([128, 1], f32, tag="v1t")
            nc.tensor.matmul(
                acc[:],
                partial[:, j * 128 : (j + 1) * 128],
                ones_col[:],
                start=True,
                stop=True,
            )
            nc.vector.tensor_add(v1T_sb[:, j : j + 1], acc[:], bvT[:, j : j + 1])

        # broadcast each v1T column across the free dim for stage-2 lhsT
        v1bc_sb = small.tile([128, D], f32)
        for k in range(KC):
            nc.vector.tensor_copy(
                v1bc_sb[:, k * 128 : (k + 1) * 128],
                v1T_sb[:, k : k + 1].broadcast_to([128, 128]),
            )

        # ---- stage 2 on PE: out_row (broadcast) = v1 @ Wo.T + bo ----
        out_sb = outp.tile([128, D], f32)
        for n in range(2):
            ns = slice(n * 512, (n + 1) * 512)
            acc = psum2.tile([128, 512], f32, tag="acc2")
            for k in range(KC):
                nc.tensor.matmul(
                    acc[:],
                    v1bc_sb[:, k * 128 : (k + 1) * 128],
                    wo_sb[:, k * D + n * 512 : k * D + (n + 1) * 512],
                    start=(k == 0),
                    stop=False,
                )
            nc.tensor.matmul(
                acc[:], ones1x128[:], bo_sb[:, ns], start=False, stop=True
            )
            nc.vector.tensor_copy(out_sb[:, ns], acc[:])

        # ---- broadcast-write the row across this core's 512 output rows ----
        for t in range(ROWS_PER_CORE // 128):
            nc.sync.dma_start(y[t * 128 : (t + 1) * 128, :], out_sb[:])

    nc.compile()
    return nc


def _prep_inputs(condition, Wv, bv, Wo, bo):
    cond = np.asarray(condition, np.float32)
    wvt = np.asarray(Wv, np.float32).T.reshape(KC, 128, D).transpose(1, 0, 2)
    wvt = wvt.reshape(128, KC * D)
    wot = np.ascontiguousarray(
        np.asarray(Wo, np.float32).T.reshape(KC, 128, D).transpose(1, 0, 2).reshape(128, KC * D)
    )
    bvT = np.asarray(bv, np.float32).reshape(KC, 128).T  # [128, KC]
    bo_ = np.ascontiguousarray(np.asarray(bo, np.float32).reshape(1, D))
    wvp = []
    for b in range(B):
        condT = cond[b].reshape(KC, 128).T  # [128, KC]
        wvp.append(np.ascontiguousarray(np.concatenate([wvt, condT, bvT], axis=1)))
    in_maps = []
    for i in range(NCORES):
        b = i // (NCORES // B)
        in_maps.append({"wvp": wvp[b], "wot": wot, "bo": bo_})
    return in_maps


def _run(in_maps, **kwargs):
    if "nc" not in _cache:
        _cache["nc"] = _build()
    return run_bass_kernel_spmd(
        _cache["nc"], in_maps, core_ids=list(range(NCORES)), **kwargs
    )


def kernel(hidden_states, condition, Wq, bq, Wk, bk, Wv, bv, Wo, bo):
    in_maps = _prep_inputs(condition, Wv, bv, Wo, bo)
    res = _run(in_maps)
    out = np.concatenate(
        [np.asarray(res.results[i]["y"]) for i in range(NCORES)], axis=0
    )
    return np.ascontiguousarray(out.reshape(B, S, D))


# revision 10
# speedup vs baseline: 1.2277x; 1.0696x over previous
"""Trainium2 Bass kernel for CrossAttentionConditionInjection.

Math: the attention keys/values come from a single condition token broadcast
across the sequence, so the scores are constant along the key axis; softmax is
exactly uniform and the attention output collapses to

    out[b, s, :] = (condition[b] @ Wv.T + bv) @ Wo.T + bo      (for every s)

independent of hidden_states / Wq / Wk / q entirely.  The kernel computes the
two 1024x1024 mat-vecs in fp32 and broadcast-writes the resulting row across
its sequence shard.

Sharding: flat (b*s) rows [4096] split into 8 shards of 512 rows; cores 0-3
serve batch 0, cores 4-7 batch 1.

Engine split (fp32 streams through the PE at ~4 cycles/column, so mat-vec
work is kept off the PE where possible):
  stage 1 (v1 = cond @ Wv.T):  DVE per-partition tensor_scalar multiplies
      (contraction dim on partitions) + tree add -> partial[128, 1024]
  partition-reduce + broadcast: PE ones-matmul  out[m,n] = sum_p partial[p,n]
      -> v1 replicated across all 128 partitions; bv added via a K=1
      rank-1 matmul with a ones row.
  stage 2 (row = v1 @ Wo.T + bo): PE broadcast-matmul over Wo.T chunks
      (accumulating over k), bias via ones-row matmul, all 128 output
      partitions = the row.
  output: 4x [128, 1024] DMA broadcast-writes.
"""

import numpy as np
from contextlib import ExitStack

import concourse.bass as bass
import concourse.bacc as bacc
import concourse.mybir as mybir
import concourse.tile as tile
from concourse.bass_utils import run_bass_kernel_spmd

B, S, D = 2, 2048, 1024
NCORES = 8
ROWS_PER_CORE = (B * S) // NCORES  # 512
KC = D // 128  # 8 contraction chunks
WO_CHUNKS = 4
WV_CHUNKS = 4

_cache = {}


def _build():
    f32 = mybir.dt.float32
    nc = bacc.Bacc()

    smalls = nc.dram_tensor("smalls", [128, 2 * KC], f32, kind="ExternalInput")
    wvp = nc.dram_tensor("wvp", [128, KC * D], f32, kind="ExternalInput")
    wot = nc.dram_tensor("wot", [128, KC * D], f32, kind="ExternalInput")
    bo = nc.dram_tensor("bo", [1, D], f32, kind="ExternalInput")
    y = nc.dram_tensor("y", [ROWS_PER_CORE, D], f32, kind="ExternalOutput")

    with tile.TileContext(nc) as tc, ExitStack() as ctx:
        wv_pool = ctx.enter_context(tc.tile_pool(name="wv", bufs=1))
        wo_pool = ctx.enter_context(tc.tile_pool(name="wo", bufs=1))
        small = ctx.enter_context(tc.tile_pool(name="small", bufs=1))
        tmpp = ctx.enter_context(tc.tile_pool(name="tmpp", bufs=1))
        outp = ctx.enter_context(tc.tile_pool(name="outp", bufs=1))
        psumv = ctx.enter_context(
            tc.tile_pool(name="psumv", bufs=2, space=bass.MemorySpace.PSUM)
        )
        psum2 = ctx.enter_context(
            tc.tile_pool(name="psum2", bufs=2, space=bass.MemorySpace.PSUM)
        )

        ones1x128 = small.tile([1, 128], f32)
        nc.vector.memset(ones1x128[:], 1.0)
        ones_col = small.tile([128, 1], f32)
        nc.vector.memset(ones_col[:], 1.0)

        # ---- loads: smalls first, then wv / wo in chunks for pipelining ----
        smalls_sb = small.tile([128, 2 * KC], f32)
        nc.sync.dma_start(smalls_sb[:], smalls[:])
        bo_sb = small.tile([1, D], f32)
        nc.sync.dma_start(bo_sb[:], bo[:])
        condT = smalls_sb[:, 0:KC]
        bvT = smalls_sb[:, KC : 2 * KC]

        wv_sb = wv_pool.tile([128, KC * D], f32)
        wv_c = (KC * D) // WV_CHUNKS
        for c in range(WV_CHUNKS):
            nc.sync.dma_start(
                wv_sb[:, c * wv_c : (c + 1) * wv_c], wvp[:, c * wv_c : (c + 1) * wv_c]
            )
        wo_sb = wo_pool.tile([128, KC * D], f32)
        wo_c = (KC * D) // WO_CHUNKS
        for c in range(WO_CHUNKS):
            nc.sync.dma_start(
                wo_sb[:, c * wo_c : (c + 1) * wo_c], wot[:, c * wo_c : (c + 1) * wo_c]
            )

        # ---- stage 1 on DVE: partial[p, c] = sum_k WvT[k*128+p, c] * cond[k*128+p]
        tmp = tmpp.tile([128, KC * D], f32)
        for k in range(KC):
            nc.vector.tensor_scalar_mul(
                tmp[:, k * D : (k + 1) * D],
                wv_sb[:, k * D : (k + 1) * D],
                condT[:, k : k + 1],
            )
        h = (KC * D) // 2
        nc.vector.tensor_add(tmp[:, :h], tmp[:, :h], tmp[:, h:])
        nc.vector.tensor_add(tmp[:, : h // 2], tmp[:, : h // 2], tmp[:, h // 2 : h])
        partial = tmp[:, :D]
        nc.vector.tensor_add(partial, partial, tmp[:, D : 2 * D])

        # ---- partition-reduce on PE, landing v1T on partitions:
        #      out[m, 0] = sum_p partial[p, j*128+m] = v1[j*128+m]
        v1T_sb = small.tile([128, KC], f32)
        for j in range(KC):
            acc = psumv.tile([128, 1], f32, tag="v1t")
            nc.tensor.matmul(
                acc[:],
                partial[:, j * 128 : (j + 1) * 128],
                ones_col[:],
                start=True,
                stop=True,
            )
            nc.vector.tensor_add(v1T_sb[:, j : j + 1], acc[:], bvT[:, j : j + 1])

        # broadcast each v1T column across the free dim for stage-2 lhsT
        v1bc_sb = small.tile([128, D], f32)
        for k in range(KC):
            nc.vector.tensor_copy(
                v1bc_sb[:, k * 128 : (k + 1) * 128],
                v1T_sb[:, k : k + 1].broadcast_to([128, 128]),
            )

        # ---- stage 2 on PE: out_row (broadcast) = v1 @ Wo.T + bo ----
        # k-outer so each wo chunk is consumed as soon as its DMA lands
        out_sb = outp.tile([128, D], f32)
        accs = [
            psum2.tile([128, 512], f32, name=f"acc2_{n}", tag=f"acc2_{n}")
            for n in range(2)
        ]
        for k in range(KC):
            for n in range(2):
                nc.tensor.matmul(
                    accs[n][:],
                    v1bc_sb[:, k * 128 : (k + 1) * 128],
                    wo_sb[:, k * D + n * 512 : k * D + (n + 1) * 512],
                    start=(k == 0),
                    stop=False,
                )
        for n in range(2):
            ns = slice(n * 512, (n + 1) * 512)
            nc.tensor.matmul(
                accs[n][:], ones1x128[:], bo_sb[:, ns], start=False, stop=True
            )
            nc.vector.tensor_copy(out_sb[:, ns], accs[n][:])

        # ---- broadcast-write the row across this core's 512 output rows ----
        for t in range(ROWS_PER_CORE // 128):
            nc.sync.dma_start(y[t * 128 : (t + 1) * 128, :], out_sb[:])

    nc.compile()
    return nc


def _prep_inputs(condition, Wv, bv, Wo, bo):
    cond = np.asarray(condition, np.float32)
    wvt = np.asarray(Wv, np.float32).T.reshape(KC, 128, D).transpose(1, 0, 2)
    wvt = wvt.reshape(128, KC * D)
    wot = np.ascontiguousarray(
        np.asarray(Wo, np.float32).T.reshape(KC, 128, D).transpose(1, 0, 2).reshape(128, KC * D)
    )
    bvT = np.asarray(bv, np.float32).reshape(KC, 128).T  # [128, KC]
    bo_ = np.ascontiguousarray(np.asarray(bo, np.float32).reshape(1, D))
    wvt = np.ascontiguousarray(wvt)
    smalls = []
    for b in range(B):
        condT = cond[b].reshape(KC, 128).T  # [128, KC]
        smalls.append(np.ascontiguousarray(np.concatenate([condT, bvT], axis=1)))
    in_maps = []
    for i in range(NCORES):
        b = i // (NCORES // B)
        in_maps.append({"smalls": smalls[b], "wvp": wvt, "wot": wot, "bo": bo_})
    return in_maps


def _run(in_maps, **kwargs):
    if "nc" not in _cache:
        _cache["nc"] = _build()
    return run_bass_kernel_spmd(
        _cache["nc"], in_maps, core_ids=list(range(NCORES)), **kwargs
    )


def kernel(hidden_states, condition, Wq, bq, Wk, bk, Wv, bv, Wo, bo):
    in_maps = _prep_inputs(condition, Wv, bv, Wo, bo)
    res = _run(in_maps)
    out = np.concatenate(
        [np.asarray(res.results[i]["y"]) for i in range(NCORES)], axis=0
    )
    return np.ascontiguousarray(out.reshape(B, S, D))


# revision 11
# speedup vs baseline: 1.2342x; 1.0053x over previous
"""Trainium2 Bass kernel for CrossAttentionConditionInjection.

Math: the attention keys/values come from a single condition token broadcast
across the sequence, so the scores are constant along the key axis; softmax is
exactly uniform and the attention output collapses to

    out[b, s, :] = (condition[b] @ Wv.T + bv) @ Wo.T + bo      (for every s)

independent of hidden_states / Wq / Wk / q entirely.  The kernel computes the
two 1024x1024 mat-vecs in fp32 and broadcast-writes the resulting row across
its sequence shard.

Sharding: flat (b*s) rows [4096] split into 8 shards of 512 rows; cores 0-3
serve batch 0, cores 4-7 batch 1.

Engine split (fp32 streams through the PE at ~4 cycles/column, so mat-vec
work is kept off the PE where possible):
  stage 1 (v1 = cond @ Wv.T):  DVE per-partition tensor_scalar multiplies
      (contraction dim on partitions) + tree add -> partial[128, 1024]
  partition-reduce + broadcast: PE ones-matmul  out[m,n] = sum_p partial[p,n]
      -> v1 replicated across all 128 partitions; bv added via a K=1
      rank-1 matmul with a ones row.
  stage 2 (row = v1 @ Wo.T + bo): PE broadcast-matmul over Wo.T chunks
      (accumulating over k), bias via ones-row matmul, all 128 output
      partitions = the row.
  output: 4x [128, 1024] DMA broadcast-writes.
"""

import numpy as np
from contextlib import ExitStack

import concourse.bass as bass
import concourse.bacc as bacc
import concourse.mybir as mybir
import concourse.tile as tile
from concourse.bass_utils import run_bass_kernel_spmd

B, S, D = 2, 2048, 1024
NCORES = 8
ROWS_PER_CORE = (B * S) // NCORES  # 512
KC = D // 128  # 8 contraction chunks
WO_CHUNKS = 4
WV_CHUNKS = 8

_cache = {}


def _build():
    f32 = mybir.dt.float32
    nc = bacc.Bacc()

    smalls = nc.dram_tensor("smalls", [128, 2 * KC], f32, kind="ExternalInput")
    wvp = nc.dram_tensor("wvp", [128, KC * D], f32, kind="ExternalInput")
    wot = nc.dram_tensor("wot", [128, KC * D], f32, kind="ExternalInput")
    bo = nc.dram_tensor("bo", [1, D], f32, kind="ExternalInput")
    y = nc.dram_tensor("y", [ROWS_PER_CORE, D], f32, kind="ExternalOutput")

    with tile.TileContext(nc) as tc, ExitStack() as ctx:
        wv_pool = ctx.enter_context(tc.tile_pool(name="wv", bufs=1))
        wo_pool = ctx.enter_context(tc.tile_pool(name="wo", bufs=1))
        small = ctx.enter_context(tc.tile_pool(name="small", bufs=1))
        tmpp = ctx.enter_context(tc.tile_pool(name="tmpp", bufs=1))
        outp = ctx.enter_context(tc.tile_pool(name="outp", bufs=1))
        psumv = ctx.enter_context(
            tc.tile_pool(name="psumv", bufs=2, space=bass.MemorySpace.PSUM)
        )
        psumw = ctx.enter_context(
            tc.tile_pool(name="psumw", bufs=1, space=bass.MemorySpace.PSUM)
        )
        psum2 = ctx.enter_context(
            tc.tile_pool(name="psum2", bufs=2, space=bass.MemorySpace.PSUM)
        )

        ones1x128 = small.tile([1, 128], f32)
        nc.vector.memset(ones1x128[:], 1.0)
        ones_col = small.tile([128, 1], f32)
        nc.vector.memset(ones_col[:], 1.0)

        # ---- loads: smalls first, then wv / wo in chunks for pipelining ----
        smalls_sb = small.tile([128, 2 * KC], f32)
        nc.sync.dma_start(smalls_sb[:], smalls[:])
        bo_sb = small.tile([1, D], f32)
        nc.sync.dma_start(bo_sb[:], bo[:])
        condT = smalls_sb[:, 0:KC]
        bvT = smalls_sb[:, KC : 2 * KC]

        wv_sb = wv_pool.tile([128, KC * D], f32)
        wv_c = (KC * D) // WV_CHUNKS
        for c in range(WV_CHUNKS):
            nc.sync.dma_start(
                wv_sb[:, c * wv_c : (c + 1) * wv_c], wvp[:, c * wv_c : (c + 1) * wv_c]
            )
        wo_sb = wo_pool.tile([128, KC * D], f32)
        wo_c = (KC * D) // WO_CHUNKS
        for c in range(WO_CHUNKS):
            nc.sync.dma_start(
                wo_sb[:, c * wo_c : (c + 1) * wo_c], wot[:, c * wo_c : (c + 1) * wo_c]
            )

        # PE warm-keepers: one tiny matmul per wv chunk arrival keeps the
        # HAM activity window from going idle so stage 2 runs at 2.4 GHz
        warm_ps = psumw.tile([128, 8], f32)
        for c in range(WV_CHUNKS):
            nc.tensor.matmul(
                warm_ps[:],
                wv_sb[:, c * ((KC * D) // WV_CHUNKS) : c * ((KC * D) // WV_CHUNKS) + 128],
                condT[:, 0:KC],
                start=True,
                stop=True,
            )

        # ---- stage 1 on DVE: partial[p, c] = sum_k WvT[k*128+p, c] * cond[k*128+p]
        # adjacent-pair add tree so each level fires as soon as its chunks land
        tmp = tmpp.tile([128, KC * D], f32)
        for k in range(KC):
            nc.vector.tensor_scalar_mul(
                tmp[:, k * D : (k + 1) * D],
                wv_sb[:, k * D : (k + 1) * D],
                condT[:, k : k + 1],
            )
        for k in range(KC // 2):  # u_k = t_{2k} + t_{2k+1}
            nc.vector.tensor_add(
                tmp[:, (2 * k) * D : (2 * k + 1) * D],
                tmp[:, (2 * k) * D : (2 * k + 1) * D],
                tmp[:, (2 * k + 1) * D : (2 * k + 2) * D],
            )
        nc.vector.tensor_add(tmp[:, 0:D], tmp[:, 0:D], tmp[:, 2 * D : 3 * D])
        nc.vector.tensor_add(tmp[:, 4 * D : 5 * D], tmp[:, 4 * D : 5 * D], tmp[:, 6 * D : 7 * D])
        partial = tmp[:, :D]
        nc.vector.tensor_add(partial, partial, tmp[:, 4 * D : 5 * D])

        # ---- partition-reduce on PE, landing v1T on partitions:
        #      out[m, 0] = sum_p partial[p, j*128+m] = v1[j*128+m]
        v1T_sb = small.tile([128, KC], f32)
        for j in range(KC):
            acc = psumv.tile([128, 1], f32, tag="v1t")
            nc.tensor.matmul(
                acc[:],
                partial[:, j * 128 : (j + 1) * 128],
                ones_col[:],
                start=True,
                stop=True,
            )
            nc.vector.tensor_add(v1T_sb[:, j : j + 1], acc[:], bvT[:, j : j + 1])

        # broadcast each v1T column across the free dim for stage-2 lhsT
        v1bc_sb = small.tile([128, D], f32)
        for k in range(KC):
            nc.vector.tensor_copy(
                v1bc_sb[:, k * 128 : (k + 1) * 128],
                v1T_sb[:, k : k + 1].broadcast_to([128, 128]),
            )

        # ---- stage 2 on PE: out_row (broadcast) = v1 @ Wo.T + bo ----
        # k-outer so each wo chunk is consumed as soon as its DMA lands
        out_sb = outp.tile([128, D], f32)
        accs = [
            psum2.tile([128, 512], f32, name=f"acc2_{n}", tag=f"acc2_{n}")
            for n in range(2)
        ]
        for k in range(KC):
            for n in range(2):
                nc.tensor.matmul(
                    accs[n][:],
                    v1bc_sb[:, k * 128 : (k + 1) * 128],
                    wo_sb[:, k * D + n * 512 : k * D + (n + 1) * 512],
                    start=(k == 0),
                    stop=False,
                )
        for n in range(2):
            ns = slice(n * 512, (n + 1) * 512)
            nc.tensor.matmul(
                accs[n][:], ones1x128[:], bo_sb[:, ns], start=False, stop=True
            )
            nc.vector.tensor_copy(out_sb[:, ns], accs[n][:])

        # ---- broadcast-write the row across this core's 512 output rows ----
        nreps = ROWS_PER_CORE // 128
        nc.sync.dma_start(
            y[:].rearrange("(a p) c -> p a c", p=128),
            out_sb[:, None, :].broadcast_to([128, nreps, D]),
        )

    nc.compile()
    return nc


def _prep_inputs(condition, Wv, bv, Wo, bo):
    cond = np.asarray(condition, np.float32)
    wvt = np.asarray(Wv, np.float32).T.reshape(KC, 128, D).transpose(1, 0, 2)
    wvt = wvt.reshape(128, KC * D)
    wot = np.ascontiguousarray(
        np.asarray(Wo, np.float32).T.reshape(KC, 128, D).transpose(1, 0, 2).reshape(128, KC * D)
    )
    bvT = np.asarray(bv, np.float32).reshape(KC, 128).T  # [128, KC]
    bo_ = np.ascontiguousarray(np.asarray(bo, np.float32).reshape(1, D))
    wvt = np.ascontiguousarray(wvt)
    smalls = []
    for b in range(B):
        condT = cond[b].reshape(KC, 128).T  # [128, KC]
        smalls.append(np.ascontiguousarray(np.concatenate([condT, bvT], axis=1)))
    in_maps = []
    for i in range(NCORES):
        b = i // (NCORES // B)
        in_maps.append({"smalls": smalls[b], "wvp": wvt, "wot": wot, "bo": bo_})
    return in_maps


def _run(in_maps, **kwargs):
    if "nc" not in _cache:
        _cache["nc"] = _build()
    return run_bass_kernel_spmd(
        _cache["nc"], in_maps, core_ids=list(range(NCORES)), **kwargs
    )


def kernel(hidden_states, condition, Wq, bq, Wk, bk, Wv, bv, Wo, bo):
    in_maps = _prep_inputs(condition, Wv, bv, Wo, bo)
    res = _run(in_maps)
    out = np.concatenate(
        [np.asarray(res.results[i]["y"]) for i in range(NCORES)], axis=0
    )
    return np.ascontiguousarray(out.reshape(B, S, D))


# revision 12
# speedup vs baseline: 1.5819x; 1.2817x over previous
"""Trainium2 Bass kernel for CrossAttentionConditionInjection.

Math: the attention keys/values come from a single condition token broadcast
across the sequence, so the scores are constant along the key axis; softmax is
exactly uniform and the attention output collapses to

    out[b, s, :] = (condition[b] @ Wv.T + bv) @ Wo.T + bo      (for every s)

independent of hidden_states / Wq / Wk / q entirely.

Sharding (2D): core i -> (batch b = i//4, output-column quarter q = i%4).
Every core computes the full v1 = cond[b] @ Wv.T + bv (Wv.T is irreducible
per-core without cross-core exchange) but only its 256-column slice of
row = v1 @ Wo.T + bo, and broadcast-writes it across all 2048 sequence
positions of its batch.  The host reassembles the column quarters.

Engine split (fp32 streams through the PE at ~4 cycles/column, so the
mat-vec bulk stays off the PE):
  stage 1 muls: ACT activation(Copy, scale=cond-per-partition), one per wv
      k-chunk, chasing the chunked wv DMA.
  stage 1 sum:  DVE rolling adds (in-place accumulate), also chasing.
  v1T:          8 tiny PE matmuls  lhsT=partial-chunk, rhs=ones column ->
                v1 landed on partitions; one DVE add folds in bv.
  v1 broadcast: single DVE copy with a step-0 AP.
  stage 2:      8 PE matmuls N=256 over the per-core Wo.T quarter + a K=1
                ones-row matmul for bo.
  output:       one DMA broadcast-writes the [128, 256] row tile 16x into
                the contiguous per-core [2048, 256] output.
"""

import numpy as np
from contextlib import ExitStack

import concourse.bass as bass
import concourse.bacc as bacc
import concourse.mybir as mybir
import concourse.tile as tile
from concourse.bass_utils import run_bass_kernel_spmd

B, S, D = 2, 2048, 1024
NCORES = 8
QCORES = NCORES // B  # cores per batch -> column quarters
QW = D // QCORES  # 256 columns per core
KC = D // 128  # 8 contraction chunks
WV_CHUNKS = 8

_cache = {}


def _build():
    f32 = mybir.dt.float32
    nc = bacc.Bacc()

    smalls = nc.dram_tensor("smalls", [128, 2 * KC], f32, kind="ExternalInput")
    wvp = nc.dram_tensor("wvp", [128, KC * D], f32, kind="ExternalInput")
    woq = nc.dram_tensor("woq", [128, KC * QW], f32, kind="ExternalInput")
    boq = nc.dram_tensor("boq", [1, QW], f32, kind="ExternalInput")
    y = nc.dram_tensor("y", [S, QW], f32, kind="ExternalOutput")

    with tile.TileContext(nc) as tc, ExitStack() as ctx:
        wv_pool = ctx.enter_context(tc.tile_pool(name="wv", bufs=1))
        wo_pool = ctx.enter_context(tc.tile_pool(name="wo", bufs=1))
        small = ctx.enter_context(tc.tile_pool(name="small", bufs=1))
        tmpp = ctx.enter_context(tc.tile_pool(name="tmpp", bufs=1))
        outp = ctx.enter_context(tc.tile_pool(name="outp", bufs=1))
        psumv = ctx.enter_context(
            tc.tile_pool(name="psumv", bufs=1, space=bass.MemorySpace.PSUM)
        )
        psum2 = ctx.enter_context(
            tc.tile_pool(name="psum2", bufs=1, space=bass.MemorySpace.PSUM)
        )

        ones1x128 = small.tile([1, 128], f32)
        nc.vector.memset(ones1x128[:], 1.0)
        ones_col = small.tile([128, 1], f32)
        nc.vector.memset(ones_col[:], 1.0)

        # ---- loads: smalls first, then wv chunked, then the wo quarter ----
        smalls_sb = small.tile([128, 2 * KC], f32)
        nc.sync.dma_start(smalls_sb[:], smalls[:])
        boq_sb = small.tile([1, QW], f32)
        nc.sync.dma_start(boq_sb[:], boq[:])
        condT = smalls_sb[:, 0:KC]
        bvT = smalls_sb[:, KC : 2 * KC]

        wv_sb = wv_pool.tile([128, KC * D], f32)
        wv_c = (KC * D) // WV_CHUNKS
        for c in range(WV_CHUNKS):
            nc.sync.dma_start(
                wv_sb[:, c * wv_c : (c + 1) * wv_c], wvp[:, c * wv_c : (c + 1) * wv_c]
            )
        woq_sb = wo_pool.tile([128, KC * QW], f32)
        nc.sync.dma_start(woq_sb[:], woq[:])

        # ---- stage 1: partial[p, c] = sum_k WvT[k*128+p, c] * cond[k*128+p]
        # ACT does the per-partition-scalar muls, DVE rolls the sum.
        tmp = tmpp.tile([128, KC * D], f32)
        for k in range(KC):
            nc.scalar.activation(
                tmp[:, k * D : (k + 1) * D],
                wv_sb[:, k * D : (k + 1) * D],
                mybir.ActivationFunctionType.Copy,
                scale=condT[:, k : k + 1],
            )
        partial = tmp[:, :D]
        for k in range(1, KC):
            nc.vector.tensor_add(partial, partial, tmp[:, k * D : (k + 1) * D])

        # ---- v1T on partitions: out[m, 0] = sum_p partial[p, j*128+m] ----
        v1T_ps = psumv.tile([128, KC], f32)
        for j in range(KC):
            nc.tensor.matmul(
                v1T_ps[:, j : j + 1],
                partial[:, j * 128 : (j + 1) * 128],
                ones_col[:],
                start=True,
                stop=True,
            )
        v1T_sb = small.tile([128, KC], f32)
        nc.vector.tensor_add(v1T_sb[:], v1T_ps[:], bvT[:])

        # broadcast each v1T column across the free dim for stage-2 lhsT
        v1bc_sb = small.tile([128, D], f32)
        nc.vector.tensor_copy(
            v1bc_sb[:].rearrange("p (k m) -> p k m", k=KC),
            v1T_sb[:, :, None].broadcast_to([128, KC, 128]),
        )

        # ---- stage 2 on PE: out quarter (broadcast) = v1 @ WoT[:, q] + bo ----
        acc = psum2.tile([128, QW], f32)
        for k in range(KC):
            nc.tensor.matmul(
                acc[:],
                v1bc_sb[:, k * 128 : (k + 1) * 128],
                woq_sb[:, k * QW : (k + 1) * QW],
                start=(k == 0),
                stop=False,
            )
        nc.tensor.matmul(acc[:], ones1x128[:], boq_sb[:], start=False, stop=True)
        out_sb = outp.tile([128, QW], f32)
        nc.vector.tensor_copy(out_sb[:], acc[:])

        # ---- broadcast-write the row quarter across all 2048 seq rows ----
        nc.sync.dma_start(
            y[:].rearrange("(a p) c -> p a c", p=128),
            out_sb[:, None, :].broadcast_to([128, S // 128, QW]),
        )

    nc.compile()
    return nc


def _prep_inputs(condition, Wv, bv, Wo, bo):
    cond = np.asarray(condition, np.float32)
    wvt = np.ascontiguousarray(
        np.asarray(Wv, np.float32).T.reshape(KC, 128, D).transpose(1, 0, 2).reshape(128, KC * D)
    )
    WoT = np.asarray(Wo, np.float32).T  # [d, dout]
    bvT = np.asarray(bv, np.float32).reshape(KC, 128).T  # [128, KC]
    bo_ = np.asarray(bo, np.float32)
    smalls = []
    for b in range(B):
        condT = cond[b].reshape(KC, 128).T  # [128, KC]
        smalls.append(np.ascontiguousarray(np.concatenate([condT, bvT], axis=1)))
    woqs, boqs = [], []
    for q in range(QCORES):
        sl = WoT[:, q * QW : (q + 1) * QW]  # [1024, 256]
        woqs.append(
            np.ascontiguousarray(
                sl.reshape(KC, 128, QW).transpose(1, 0, 2).reshape(128, KC * QW)
            )
        )
        boqs.append(np.ascontiguousarray(bo_[q * QW : (q + 1) * QW].reshape(1, QW)))
    in_maps = []
    for i in range(NCORES):
        b, q = i // QCORES, i % QCORES
        in_maps.append(
            {"smalls": smalls[b], "wvp": wvt, "woq": woqs[q], "boq": boqs[q]}
        )
    return in_maps


def _run(in_maps, **kwargs):
    if "nc" not in _cache:
        _cache["nc"] = _build()
    return run_bass_kernel_spmd(
        _cache["nc"], in_maps, core_ids=list(range(NCORES)), **kwargs
    )


def kernel(hidden_states, condition, Wq, bq, Wk, bk, Wv, bv, Wo, bo):
    in_maps = _prep_inputs(condition, Wv, bv, Wo, bo)
    res = _run(in_maps)
    full = np.empty((B, S, D), np.float32)
    for i in range(NCORES):
        b, q = i // QCORES, i % QCORES
        full[b, :, q * QW : (q + 1) * QW] = np.asarray(res.results[i]["y"])
    return full
